# revision 1
# baseline (speedup 1.0000x reference)
"""CorrectedNeoDTI GNN message passing on 8 TRN2 NeuronCores.

Strategy (graph-partition parallelism, dst-partitioned):
  - Host: sort each relation's edges by destination, partition destinations
    into 8 contiguous ranges with balanced edge counts, and pack each core's
    destinations into "chunks" of <=128 dsts / bounded edge count.  Node
    tables are remapped into a padded layout where chunk k of core c owns
    rows [c*S + k*128, c*S + (k+1)*128).
  - Device (SPMD, identical program, per-core data): per chunk, dma_gather
    the source rows (bf16, 256B rows; the drug table exceeds int16 index
    range so its chunks use two gathers, low/high half), build one-hot
    segment matrices on DVE (iota == dstrel, scaled by 1/deg), accumulate
    aggT = sum_e X[e,:]^T one_hot[e,:] on the TensorEngine into PSUM.
    Then Y = (aggT)^T @ W^T via a second matmul, fused residual + LayerNorm +
    ReLU epilogue, writing the core's updated slab.
  - AllGather slabs between layers to rebuild the full (padded) tables.
  - Readout: each core computes masked partial dot products for its owned
    rows; a tiny AllReduce + sigmoid produces the final [4096] output.
"""

import sys
from contextlib import ExitStack

sys.path.insert(0, "/opt/trn_rl_repo")

import numpy as np
import ml_dtypes

import concourse.bacc as bacc
import concourse.tile as tile
import concourse.mybir as mybir
from concourse.bass_utils import run_bass_kernel_spmd

BF16 = ml_dtypes.bfloat16
F32 = np.float32

NCORES = 8
H = 128
P = 128
NUM_DRUG, NUM_CELL, NE, NB = 50000, 20000, 600000, 4096
EPS = 1e-5
CH_DT = 28   # edge-slot tiles per chunk, dt relation (dst=cell, avg deg 30)
CH_TD = 12   # td relation (dst=drug, avg deg 12)
SPLIT = 32768  # int16 index ceiling for dma_gather
STAGE = 5    # debug: truncate program after stage N (5 = full)
RO_J = NB // P


def wrap_idx16(flat_idx):
    """[n] int -> [128, n//16] int16 in dma_gather's 16-partition wrapped
    layout, replicated across all 8 GPSIMD core groups (the ucode on each
    Q7 core reads indices from its own 16-partition group)."""
    n = flat_idx.shape[0]
    assert n % 16 == 0
    blk = flat_idx.reshape(n // 16, 16).T.astype(np.int16)
    return np.tile(blk, (8, 1))


# ----------------------------------------------------------------------------
# Host-side graph preprocessing
# ----------------------------------------------------------------------------

class RelPrep:
    """Per-relation chunked CSR in padded-slab layout."""

    def __init__(self, src, dst, n_dst, ch_t):
        src = np.asarray(src, dtype=np.int64)
        dst = np.asarray(dst, dtype=np.int64)
        ne = src.shape[0]
        self.ch_t = ch_t
        cnt = np.bincount(dst, minlength=n_dst)
        order = np.argsort(dst, kind="stable")
        self.src_sorted = src[order]
        self.dst_sorted = dst[order]
        self.dst_starts = np.zeros(n_dst + 1, dtype=np.int64)
        np.cumsum(cnt, out=self.dst_starts[1:])

        bounds = [0]
        for c in range(1, NCORES):
            target = ne * c // NCORES
            bounds.append(int(np.searchsorted(self.dst_starts, target)))
        bounds.append(n_dst)

        cap = ch_t * P
        self.per_core_chunks = []
        for c in range(NCORES):
            d_lo, d_hi = bounds[c], bounds[c + 1]
            chunks = []
            cur_lo = d_lo
            cur_edges = 0
            for d in range(d_lo, d_hi):
                deg = int(cnt[d])
                assert deg <= cap
                if (d - cur_lo) == P or cur_edges + deg > cap:
                    chunks.append((cur_lo, d))
                    cur_lo, cur_edges = d, 0
                cur_edges += deg
            if cur_lo < d_hi:
                chunks.append((cur_lo, d_hi))
            self.per_core_chunks.append(chunks)

        self.C = max(len(ch) for ch in self.per_core_chunks)
        self.S = (self.C + 1) * P  # +1 guaranteed-zero pad block
        self.cnt = cnt
        self.recip = 1.0 / np.maximum(cnt, 1).astype(F32)

        self.remap = np.zeros(n_dst, dtype=np.int64)
        for c in range(NCORES):
            for k, (a, b) in enumerate(self.per_core_chunks[c]):
                self.remap[a:b] = c * self.S + k * P + np.arange(b - a)

    def finalize(self, remap_src, split):
        """Build device arrays. remap_src maps original src ids to padded
        source-table rows; split = None (single gather per chunk) or the
        int16 row ceiling (two gathers per chunk, low/high)."""
        if split is not None:
            t_lo = t_hi = 1
            for c in range(NCORES):
                for (a, b) in self.per_core_chunks[c]:
                    r = remap_src[self.src_sorted[self.dst_starts[a]:
                                                  self.dst_starts[b]]]
                    nlo = int((r < split).sum())
                    nhi = r.shape[0] - nlo
                    t_lo = max(t_lo, -(-nlo // P))
                    t_hi = max(t_hi, -(-nhi // P))
            self.t_lo, self.t_hi = t_lo, t_hi
            ch3 = t_lo + t_hi
        else:
            self.t_lo, self.t_hi = self.ch_t, 0
            ch3 = self.ch_t
        self.ch3 = ch3
        self.T3 = self.C * ch3

        nslots = self.T3 * P
        self.idx16 = np.zeros((NCORES, P, nslots // 16), dtype=np.int16)
        self.dstrel = np.full((NCORES, P, self.T3), -1.0, dtype=F32)
        self.recipe = np.zeros((NCORES, P, self.T3), dtype=F32)
        self.plain_idx = np.zeros((NCORES, nslots), dtype=np.int64)

        for c in range(NCORES):
            idx_flat = np.zeros(nslots, dtype=np.int64)   # biased table rows
            plain = np.zeros(nslots, dtype=np.int64)       # unbiased rows
            rel_flat = np.full(nslots, -1.0, dtype=F32)
            rec_flat = np.zeros(nslots, dtype=F32)
            for k, (a, b) in enumerate(self.per_core_chunks[c]):
                e_lo, e_hi = self.dst_starts[a], self.dst_starts[b]
                r = remap_src[self.src_sorted[e_lo:e_hi]]
                dd = (self.dst_sorted[e_lo:e_hi] - a).astype(F32)
                rc = self.recip[self.dst_sorted[e_lo:e_hi]]
                base = k * ch3 * P
                if split is not None:
                    lo_m = r < split
                    for sel, off, boff in ((lo_m, 0, 0),
                                           (~lo_m, self.t_lo * P, split)):
                        rr = r[sel]
                        n = rr.shape[0]
                        sl = slice(base + off, base + off + n)
                        idx_flat[sl] = rr - boff
                        plain[sl] = rr
                        rel_flat[sl] = dd[sel]
                        rec_flat[sl] = rc[sel]
                        # high-half padding must index within the high slice
                else:
                    n = r.shape[0]
                    sl = slice(base, base + n)
                    idx_flat[sl] = r
                    plain[sl] = r
                    rel_flat[sl] = dd
                    rec_flat[sl] = rc
            self.idx16[c] = wrap_idx16(idx_flat)
            self.plain_idx[c] = plain
            self.dstrel[c] = rel_flat.reshape(self.T3, P).T
            self.recipe[c] = rec_flat.reshape(self.T3, P).T


def _pad_table(emb, remap, s_total):
    out = np.zeros((s_total, H), dtype=BF16)
    out[remap] = np.asarray(emb, dtype=F32).astype(BF16)
    return out


# ----------------------------------------------------------------------------
# Device program
# ----------------------------------------------------------------------------

class _StageDone(Exception):
    pass


def build_program(c_dt, c_td, ch3_dt, tlo_dt, ch3_td):
    s_cell, s_drug = (c_dt + 1) * P, (c_td + 1) * P
    dt_bf, dt_f32, dt_i16 = mybir.dt.bfloat16, mybir.dt.float32, mybir.dt.int16
    assert NCORES * s_cell <= SPLIT, "cell table exceeds int16 gather range"

    nc = bacc.Bacc("TRN2", target_bir_lowering=False, debug=False,
                   enable_asserts=False, num_devices=NCORES)

    def ein(name, shape, dtype):
        return nc.dram_tensor(name, shape, dtype, kind="ExternalInput")

    tbl_drug1 = ein("tbl_drug", [NCORES * s_drug, H], dt_bf)
    tbl_cell1 = ein("tbl_cell", [NCORES * s_cell, H], dt_bf)
    slab0_drug = ein("slab0_drug", [s_drug, H], dt_bf)
    slab0_cell = ein("slab0_cell", [s_cell, H], dt_bf)
    idx_dt_d = ein("idx_dt", [P, c_dt * ch3_dt * 8], dt_i16)
    rel_dt_d = ein("rel_dt", [P, c_dt * ch3_dt], dt_f32)
    rec_dt_d = ein("rec_dt", [P, c_dt * ch3_dt], dt_f32)
    idx_td_d = ein("idx_td", [P, c_td * ch3_td * 8], dt_i16)
    rel_td_d = ein("rel_td", [P, c_td * ch3_td], dt_f32)
    rec_td_d = ein("rec_td", [P, c_td * ch3_td], dt_f32)
    w_dt_t_d = ein("w_dt_t", [H, H], dt_bf)
    w_td_t_d = ein("w_td_t", [H, H], dt_bf)
    iota_d = ein("iota", [P, P], dt_bf)
    wd_b_d = ein("wd_b", [P, H], dt_bf)
    wc_b_d = ein("wc_b", [P, H], dt_bf)
    wfb_d = ein("wfb", [P, 1], dt_f32)
    ro_d_d = ein("ro_d_idx", [P, RO_J * 8], dt_i16)
    ro_c_d = ein("ro_c_idx", [P, RO_J * 8], dt_i16)
    out_d = nc.dram_tensor("out", [P, RO_J], dt_f32, kind="ExternalOutput")
    tbl_cell2 = nc.dram_tensor("tbl_cell2", [NCORES * s_cell, H], dt_bf,
                               addr_space="Shared")
    tbl_drug2 = nc.dram_tensor("tbl_drug2", [NCORES * s_drug, H], dt_bf,
                               addr_space="Shared")
    ar_out = nc.dram_tensor("ar_out", [P, RO_J], dt_f32, addr_space="Shared")

    with tile.TileContext(nc) as tc, ExitStack() as stk:
      def _body():
        consts = stk.enter_context(tc.tile_pool(name="consts", bufs=1))

        def load_const(dram, shape, dtype, name):
            t = consts.tile(shape, dtype, tag=name)
            nc.sync.dma_start(out=t[:], in_=dram[:, :])
            return t

        idx_dt = load_const(idx_dt_d, [P, c_dt * ch3_dt * 8], dt_i16, "idx_dt")
        rel_dt = load_const(rel_dt_d, [P, c_dt * ch3_dt], dt_f32, "rel_dt")
        rec_dt = load_const(rec_dt_d, [P, c_dt * ch3_dt], dt_f32, "rec_dt")
        idx_td = load_const(idx_td_d, [P, c_td * ch3_td * 8], dt_i16, "idx_td")
        rel_td = load_const(rel_td_d, [P, c_td * ch3_td], dt_f32, "rel_td")
        rec_td = load_const(rec_td_d, [P, c_td * ch3_td], dt_f32, "rec_td")
        w_dt_t = load_const(w_dt_t_d, [H, H], dt_bf, "w_dt_t")
        w_td_t = load_const(w_td_t_d, [H, H], dt_bf, "w_td_t")
        iota = load_const(iota_d, [P, P], dt_bf, "iota")
        wd_b = load_const(wd_b_d, [P, H], dt_bf, "wd_b")
        wc_b = load_const(wc_b_d, [P, H], dt_bf, "wc_b")
        wfb = load_const(wfb_d, [P, 1], dt_f32, "wfb")
        eps_t = consts.tile([P, 1], dt_f32, tag="eps_t")
        nc.vector.memset(eps_t[:], EPS)

        def load_slab(dram, C, name):
            t = consts.tile([P, (C + 1) * P], dt_bf, tag=name)
            nc.sync.dma_start(
                out=t[:].rearrange("p (k f) -> p k f", f=H),
                in_=dram[:, :].rearrange("(k p) f -> k p f", p=P)
                .rearrange("k p f -> p k f"),
            )
            return t

        slab_cell_old = load_slab(slab0_cell, c_dt, "slab_cell_old")
        slab_drug_old = load_slab(slab0_drug, c_td, "slab_drug_old")

        work = stk.enter_context(tc.tile_pool(name="work", bufs=4))
        spool = stk.enter_context(tc.tile_pool(name="spool", bufs=6))
        psum = stk.enter_context(tc.tile_pool(name="psum", bufs=3, space="PSUM"))
        epil = stk.enter_context(tc.tile_pool(name="epil", bufs=2))

        def relation_pass(tbl_ap, idx_sb, rel_sb, rec_sb, w_t_sb, slab_old,
                          C, ch3, t_lo, split_rows, tag):
            """One message-passing direction; returns SBUF slab_new (bf16)."""
            sx_tag = "sx_dt" if tag[0] == "c" else "sx_td"
            slab_x = consts.tile([P, C * P], dt_f32, tag=sx_tag)
            xsum = consts.tile([P, C], dt_f32, tag=f"xsum_{tag}")
            ssq = consts.tile([P, C], dt_f32, tag=f"ssq_{tag}")
            slab_new = consts.tile([P, (C + 1) * P], dt_bf, tag=f"snew_{tag}")
            nc.vector.memset(slab_new[:, C * P:(C + 1) * P], 0)

            t_hi = ch3 - t_lo
            for k in range(C):
                xg = work.tile([P, ch3 * P], dt_bf, tag="xg")
                xg3 = xg[:].rearrange("p (t f) -> p t f", f=H)
                ib = k * ch3 * 8
                if split_rows is None:
                    nc.gpsimd.dma_gather(
                        xg3, tbl_ap, idx_sb[:, ib:ib + ch3 * 8],
                        ch3 * P, ch3 * P, H, single_packet=False)
                else:
                    nc.gpsimd.dma_gather(
                        xg3[:, :t_lo, :], tbl_ap[:split_rows, :],
                        idx_sb[:, ib:ib + t_lo * 8],
                        t_lo * P, t_lo * P, H, single_packet=False)
                    nc.gpsimd.dma_gather(
                        xg3[:, t_lo:, :], tbl_ap[split_rows:, :],
                        idx_sb[:, ib + t_lo * 8:ib + ch3 * 8],
                        t_hi * P, t_hi * P, H, single_packet=False)
                aggt = psum.tile([P, P], dt_f32, tag="aggt")
                for j in range(ch3):
                    t = k * ch3 + j
                    s_t = spool.tile([P, P], dt_bf, tag="s_t")
                    nc.vector.tensor_scalar(
                        out=s_t[:], in0=iota[:],
                        scalar1=rel_sb[:, t:t + 1],
                        scalar2=rec_sb[:, t:t + 1],
                        op0=mybir.AluOpType.is_equal,
                        op1=mybir.AluOpType.mult)
                    nc.tensor.matmul(
                        out=aggt[:], lhsT=xg[:, j * P:(j + 1) * P],
                        rhs=s_t[:], start=(j == 0), stop=(j == ch3 - 1))
                aggt_sb = work.tile([P, P], dt_bf, tag="aggt_sb")
                nc.vector.tensor_copy(out=aggt_sb[:], in_=aggt[:])
                y_ps = psum.tile([P, P], dt_f32, tag="y_ps")
                nc.tensor.matmul(out=y_ps[:], lhsT=aggt_sb[:], rhs=w_t_sb[:],
                                 start=True, stop=True)
                nc.vector.scalar_tensor_tensor(
                    out=slab_x[:, k * P:(k + 1) * P], in0=y_ps[:], scalar=1.0,
                    in1=slab_old[:, k * P:(k + 1) * P],
                    op0=mybir.AluOpType.mult, op1=mybir.AluOpType.add,
                    accum_out=xsum[:, k:k + 1])
                sq_tr = epil.tile([P, P], dt_f32, tag="sq_tr")
                nc.scalar.activation(
                    out=sq_tr[:], in_=slab_x[:, k * P:(k + 1) * P],
                    func=mybir.ActivationFunctionType.Square,
                    accum_out=ssq[:, k:k + 1])

            mu_neg = consts.tile([P, C], dt_f32, tag=f"mneg_{tag}")
            nc.vector.tensor_scalar(
                out=mu_neg[:], in0=xsum[:], scalar1=-1.0 / H, scalar2=None,
                op0=mybir.AluOpType.mult)
            mu2 = consts.tile([P, C], dt_f32, tag=f"mu2_{tag}")
            nc.vector.tensor_tensor(out=mu2[:], in0=mu_neg[:], in1=mu_neg[:],
                                    op=mybir.AluOpType.mult)
            var = consts.tile([P, C], dt_f32, tag=f"var_{tag}")
            nc.vector.scalar_tensor_tensor(
                out=var[:], in0=ssq[:], scalar=1.0 / H, in1=mu2[:],
                op0=mybir.AluOpType.mult, op1=mybir.AluOpType.subtract)
            std = consts.tile([P, C], dt_f32, tag=f"std_{tag}")
            nc.scalar.activation(out=std[:], in_=var[:],
                                 func=mybir.ActivationFunctionType.Sqrt,
                                 bias=eps_t[:, :1])
            rstd = consts.tile([P, C], dt_f32, tag=f"rstd_{tag}")
            nc.vector.reciprocal(out=rstd[:], in_=std[:])
            bvec = consts.tile([P, C], dt_f32, tag=f"bvec_{tag}")
            nc.vector.tensor_tensor(out=bvec[:], in0=mu_neg[:], in1=rstd[:],
                                    op=mybir.AluOpType.mult)
            for k in range(C):
                nc.scalar.activation(
                    out=slab_new[:, k * P:(k + 1) * P],
                    in_=slab_x[:, k * P:(k + 1) * P],
                    func=mybir.ActivationFunctionType.Relu,
                    bias=bvec[:, k:k + 1], scale=rstd[:, k:k + 1])
            return slab_new

        def slab_to_dram(slab_sb, C, name):
            d, _f = tc.tile([(C + 1) * P, H], dt_bf, space="DRAM", name=name)
            stk.callback(_f)
            nc.sync.dma_start(
                out=d[:].rearrange("(k p) f -> k p f", p=P)
                .rearrange("k p f -> p k f"),
                in_=slab_sb[:].rearrange("p (k f) -> p k f", f=H))
            return d

        rg = [list(range(NCORES))]
        dt_split = SPLIT if tlo_dt < ch3_dt else None

        def dbg_out(slab_sb):
            probe = consts.tile([P, RO_J], dt_f32, tag="probe")
            nc.vector.tensor_copy(out=probe[:], in_=slab_sb[:, :RO_J])
            nc.sync.dma_start(out=out_d[:, :], in_=probe[:])

        # ---- layer 1 ----
        slab_cell_1 = relation_pass(tbl_drug1[:, :], idx_dt, rel_dt, rec_dt,
                                    w_dt_t, slab_cell_old, c_dt, ch3_dt,
                                    tlo_dt, dt_split, "c1")
        if STAGE == 1:
            dbg_out(slab_cell_1)
        if STAGE < 2:
            return
        slab_cell_1d = slab_to_dram(slab_cell_1, c_dt, "slab_cell_1d")
        nc.gpsimd.collective_compute(
            "AllGather", mybir.AluOpType.bypass, replica_groups=rg,
            ins=[slab_cell_1d[:]], outs=[tbl_cell2[:, :]])

        slab_drug_1 = relation_pass(tbl_cell1[:, :], idx_td, rel_td, rec_td,
                                    w_td_t, slab_drug_old, c_td, ch3_td,
                                    ch3_td, None, "d1")
        if STAGE == 2:
            dbg_out(slab_drug_1)
        if STAGE < 3:
            return

        slab_drug_1d = slab_to_dram(slab_drug_1, c_td, "slab_drug_1d")
        nc.gpsimd.collective_compute(
            "AllGather", mybir.AluOpType.bypass, replica_groups=rg,
            ins=[slab_drug_1d[:]], outs=[tbl_drug2[:, :]])

        if STAGE == 3:
            probe_b = consts.tile([P, RO_J], dt_bf, tag="probe_b")
            nc.sync.dma_start(out=probe_b[:], in_=tbl_cell2[0:P, 0:RO_J])
            dbg_out(probe_b)
        if STAGE < 4:
            return

        # ---- layer 2 (td first: needs the cell table, gathered first) ----
        slab_drug_2 = relation_pass(tbl_cell2[:, :], idx_td, rel_td, rec_td,
                                    w_td_t, slab_drug_1, c_td, ch3_td,
                                    ch3_td, None, "d2")
        slab_cell_2 = relation_pass(tbl_drug2[:, :], idx_dt, rel_dt, rec_dt,
                                    w_dt_t, slab_cell_1, c_dt, ch3_dt,
                                    tlo_dt, dt_split, "c2")

        if STAGE == 4:
            dbg_out(slab_drug_2)
        if STAGE < 5:
            return

        # ---- readout ----
        slab_drug_2d = slab_to_dram(slab_drug_2, c_td, "slab_drug_2d")
        slab_cell_2d = slab_to_dram(slab_cell_2, c_dt, "slab_cell_2d")

        ro_d = load_const(ro_d_d, [P, RO_J * 8], dt_i16, "ro_d")
        ro_c = load_const(ro_c_d, [P, RO_J * 8], dt_i16, "ro_c")

        xd = consts.tile([P, RO_J * H], dt_bf, tag="xd")
        nc.gpsimd.dma_gather(
            xd[:].rearrange("p (j f) -> p j f", f=H), slab_drug_2d[:],
            ro_d[:], NB, NB, H, single_packet=False)
        xc = consts.tile([P, RO_J * H], dt_bf, tag="xc")
        nc.gpsimd.dma_gather(
            xc[:].rearrange("p (j f) -> p j f", f=H), slab_cell_2d[:],
            ro_c[:], NB, NB, H, single_packet=False)

        sd = consts.tile([P, RO_J], dt_f32, tag="sd")
        sc = consts.tile([P, RO_J], dt_f32, tag="sc")
        for j in range(RO_J):
            for x_t, w_t, s_t in ((xd, wd_b, sd), (xc, wc_b, sc)):
                mres = epil.tile([P, H], dt_f32, tag="ro_mres")
                nc.vector.tensor_tensor(out=mres[:],
                                        in0=x_t[:, j * H:(j + 1) * H],
                                        in1=w_t[:], op=mybir.AluOpType.mult)
                nc.vector.tensor_reduce(out=s_t[:, j:j + 1], in_=mres[:],
                                        axis=mybir.AxisListType.X,
                                        op=mybir.AluOpType.add)
        part = consts.tile([P, RO_J], dt_f32, tag="part")
        nc.vector.tensor_tensor(out=part[:], in0=sd[:], in1=sc[:],
                                op=mybir.AluOpType.add)
        ar_in, _f3 = tc.tile([P, RO_J], dt_f32, space="DRAM", name="ar_in")
        stk.callback(_f3)
        nc.sync.dma_start(out=ar_in[:], in_=part[:])
        nc.gpsimd.collective_compute(
            "AllReduce", mybir.AluOpType.add, replica_groups=rg,
            ins=[ar_in[:]], outs=[ar_out[:, :]])
        logit = consts.tile([P, RO_J], dt_f32, tag="logit")
        nc.sync.dma_start(out=logit[:], in_=ar_out[:, :])
        prob = consts.tile([P, RO_J], dt_f32, tag="prob")
        nc.scalar.activation(out=prob[:], in_=logit[:],
                             func=mybir.ActivationFunctionType.Sigmoid,
                             bias=wfb[:, :1])
        nc.sync.dma_start(out=out_d[:, :], in_=prob[:])
      _body()

    nc.compile()
    return nc


# ----------------------------------------------------------------------------
# Entry point
# ----------------------------------------------------------------------------

_CACHE = {}


def _prepare(inputs):
    dt = RelPrep(inputs["edge_dt_src"], inputs["edge_dt_dst"], NUM_CELL, CH_DT)
    td = RelPrep(inputs["edge_td_src"], inputs["edge_td_dst"], NUM_DRUG, CH_TD)
    # dt gathers drug rows (laid out by td's chunks); td gathers cell rows
    dt.finalize(td.remap, SPLIT if NCORES * td.S > SPLIT else None)
    td.finalize(dt.remap, SPLIT if NCORES * dt.S > SPLIT else None)

    tbl_drug = _pad_table(inputs["emb_drug"], td.remap, NCORES * td.S)
    tbl_cell = _pad_table(inputs["emb_cell"], dt.remap, NCORES * dt.S)

    w_dt_t = np.ascontiguousarray(np.asarray(inputs["W_dt"], dtype=F32).T).astype(BF16)
    w_td_t = np.ascontiguousarray(np.asarray(inputs["W_td"], dtype=F32).T).astype(BF16)
    iota = np.tile(np.arange(P, dtype=F32), (P, 1)).astype(BF16)
    wf = np.asarray(inputs["W_final_w"], dtype=F32)[0]
    wd_b = np.tile(wf[:H], (P, 1)).astype(BF16)
    wc_b = np.tile(wf[H:], (P, 1)).astype(BF16)
    wfb = np.full((P, 1), np.asarray(inputs["W_final_b"], dtype=F32)[0], dtype=F32)

    drug_ids = np.asarray(inputs["drug_ids"], dtype=np.int64)
    cell_ids = np.asarray(inputs["cell_ids"], dtype=np.int64)
    gd = td.remap[drug_ids].reshape(P, RO_J)
    gc = dt.remap[cell_ids].reshape(P, RO_J)

    in_maps = []
    for c in range(NCORES):
        own_d = (gd // td.S) == c
        own_c = (gc // dt.S) == c
        ro_d = np.where(own_d, gd - c * td.S, td.C * P)
        ro_c = np.where(own_c, gc - c * dt.S, dt.C * P)
        # dma_gather puts flat position i at dst[i%128, i//128]; batch element
        # b = p*RO_J + j must land at [p, j] => use position i = j*128 + p.
        ro_d_flat = ro_d.T.reshape(-1)
        ro_c_flat = ro_c.T.reshape(-1)
        in_maps.append({
            "tbl_drug": tbl_drug,
            "tbl_cell": tbl_cell,
            "slab0_drug": tbl_drug[c * td.S:(c + 1) * td.S],
            "slab0_cell": tbl_cell[c * dt.S:(c + 1) * dt.S],
            "idx_dt": dt.idx16[c],
            "rel_dt": dt.dstrel[c].astype(F32),
            "rec_dt": dt.recipe[c].astype(F32),
            "idx_td": td.idx16[c],
            "rel_td": td.dstrel[c].astype(F32),
            "rec_td": td.recipe[c].astype(F32),
            "w_dt_t": w_dt_t,
            "w_td_t": w_td_t,
            "iota": iota,
            "wd_b": wd_b,
            "wc_b": wc_b,
            "wfb": wfb,
            "ro_d_idx": wrap_idx16(ro_d_flat),
            "ro_c_idx": wrap_idx16(ro_c_flat),
        })
    return dt, td, in_maps


def kernel(**inputs) -> np.ndarray:
    dt, td, in_maps = _prepare(inputs)
    key = (dt.C, td.C, dt.ch3, dt.t_lo, td.ch3)
    if key not in _CACHE:
        _CACHE[key] = build_program(dt.C, td.C, dt.ch3, dt.t_lo, td.ch3)
    nc = _CACHE[key]
    res = run_bass_kernel_spmd(nc, in_maps, core_ids=list(range(NCORES)))
    out = res.results[0]["out"]
    return np.asarray(out, dtype=np.float32).reshape(NB)



# revision 3
# speedup vs baseline: 389.2047x; 389.2047x over previous
"""CorrectedNeoDTI GNN message passing on 8 TRN2 NeuronCores.

Strategy (graph-partition parallelism, dst-partitioned):
  - Host: sort each relation's edges by destination, partition destinations
    into 8 contiguous ranges with balanced edge counts, and pack each core's
    destinations into "chunks" of <=128 dsts / bounded edge count.  Node
    tables are remapped into a padded layout where chunk k of core c owns
    rows [c*S + k*128, c*S + (k+1)*128).
  - Device (SPMD, identical program, per-core data): per chunk, dma_gather
    the source rows (bf16, 256B rows; the drug table exceeds int16 index
    range so its chunks use two gathers, low/high half), build one-hot
    segment matrices on DVE (iota == dstrel, scaled by 1/deg), accumulate
    aggT = sum_e X[e,:]^T one_hot[e,:] on the TensorEngine into PSUM.
    Then Y = (aggT)^T @ W^T via a second matmul, fused residual + LayerNorm +
    ReLU epilogue, writing the core's updated slab.
  - AllGather slabs between layers to rebuild the full (padded) tables.
  - Readout: each core computes masked partial dot products for its owned
    rows; a tiny AllReduce + sigmoid produces the final [4096] output.
"""

import sys
from contextlib import ExitStack

sys.path.insert(0, "/opt/trn_rl_repo")

import numpy as np
import ml_dtypes

import concourse.bacc as bacc
import concourse.tile as tile
import concourse.mybir as mybir
from concourse.bass_utils import run_bass_kernel_spmd

BF16 = ml_dtypes.bfloat16
F32 = np.float32

NCORES = 8
H = 128
P = 128
NUM_DRUG, NUM_CELL, NE, NB = 50000, 20000, 600000, 4096
EPS = 1e-5
CH_DT = 28   # edge-slot tiles per chunk, dt relation (dst=cell, avg deg 30)
CH_TD = 12   # td relation (dst=drug, avg deg 12)
SPLIT = 32768  # int16 index ceiling for dma_gather
STAGE = 5    # debug: truncate program after stage N (5 = full)
RO_J = NB // P


def wrap_idx16(flat_idx):
    """[n] int -> [128, n//16] int16 in dma_gather's 16-partition wrapped
    layout, replicated across all 8 GPSIMD core groups (the ucode on each
    Q7 core reads indices from its own 16-partition group)."""
    n = flat_idx.shape[0]
    assert n % 16 == 0
    blk = flat_idx.reshape(n // 16, 16).T.astype(np.int16)
    return np.tile(blk, (8, 1))


# ----------------------------------------------------------------------------
# Host-side graph preprocessing
# ----------------------------------------------------------------------------

class RelPrep:
    """Per-relation chunked CSR in padded-slab layout."""

    def __init__(self, src, dst, n_dst, ch_t):
        src = np.asarray(src, dtype=np.int64)
        dst = np.asarray(dst, dtype=np.int64)
        ne = src.shape[0]
        self.ch_t = ch_t
        cnt = np.bincount(dst, minlength=n_dst)
        order = np.argsort(dst, kind="stable")
        self.src_sorted = src[order]
        self.dst_sorted = dst[order]
        self.dst_starts = np.zeros(n_dst + 1, dtype=np.int64)
        np.cumsum(cnt, out=self.dst_starts[1:])

        bounds = [0]
        for c in range(1, NCORES):
            target = ne * c // NCORES
            bounds.append(int(np.searchsorted(self.dst_starts, target)))
        bounds.append(n_dst)

        cap = ch_t * P
        self.per_core_chunks = []
        for c in range(NCORES):
            d_lo, d_hi = bounds[c], bounds[c + 1]
            chunks = []
            cur_lo = d_lo
            cur_edges = 0
            for d in range(d_lo, d_hi):
                deg = int(cnt[d])
                assert deg <= cap
                if (d - cur_lo) == P or cur_edges + deg > cap:
                    chunks.append((cur_lo, d))
                    cur_lo, cur_edges = d, 0
                cur_edges += deg
            if cur_lo < d_hi:
                chunks.append((cur_lo, d_hi))
            self.per_core_chunks.append(chunks)

        self.C = max(len(ch) for ch in self.per_core_chunks)
        self.S = (self.C + 1) * P  # +1 guaranteed-zero pad block
        self.cnt = cnt
        self.recip = 1.0 / np.maximum(cnt, 1).astype(F32)

        self.remap = np.zeros(n_dst, dtype=np.int64)
        for c in range(NCORES):
            for k, (a, b) in enumerate(self.per_core_chunks[c]):
                self.remap[a:b] = c * self.S + k * P + np.arange(b - a)

    def finalize(self, remap_src, split):
        """Build device arrays. remap_src maps original src ids to padded
        source-table rows; split = None (single gather per chunk) or the
        int16 row ceiling (two gathers per chunk, low/high)."""
        if split is not None:
            t_lo = t_hi = 1
            for c in range(NCORES):
                for (a, b) in self.per_core_chunks[c]:
                    r = remap_src[self.src_sorted[self.dst_starts[a]:
                                                  self.dst_starts[b]]]
                    nlo = int((r < split).sum())
                    nhi = r.shape[0] - nlo
                    t_lo = max(t_lo, -(-nlo // P))
                    t_hi = max(t_hi, -(-nhi // P))
            self.t_lo, self.t_hi = t_lo, t_hi
            ch3 = t_lo + t_hi
        else:
            self.t_lo, self.t_hi = self.ch_t, 0
            ch3 = self.ch_t
        self.ch3 = ch3
        self.T3 = self.C * ch3

        nslots = self.T3 * P
        self.idx16 = np.zeros((NCORES, P, nslots // 16), dtype=np.int16)
        self.dstrel = np.full((NCORES, P, self.T3), -1.0, dtype=F32)
        self.recipe = np.zeros((NCORES, P, self.T3), dtype=F32)
        self.plain_idx = np.zeros((NCORES, nslots), dtype=np.int64)

        for c in range(NCORES):
            idx_flat = np.zeros(nslots, dtype=np.int64)   # biased table rows
            plain = np.zeros(nslots, dtype=np.int64)       # unbiased rows
            rel_flat = np.full(nslots, -1.0, dtype=F32)
            rec_flat = np.zeros(nslots, dtype=F32)
            for k, (a, b) in enumerate(self.per_core_chunks[c]):
                e_lo, e_hi = self.dst_starts[a], self.dst_starts[b]
                r = remap_src[self.src_sorted[e_lo:e_hi]]
                dd = (self.dst_sorted[e_lo:e_hi] - a).astype(F32)
                rc = self.recip[self.dst_sorted[e_lo:e_hi]]
                base = k * ch3 * P
                if split is not None:
                    lo_m = r < split
                    for sel, off, boff in ((lo_m, 0, 0),
                                           (~lo_m, self.t_lo * P, split)):
                        rr = r[sel]
                        n = rr.shape[0]
                        sl = slice(base + off, base + off + n)
                        idx_flat[sl] = rr - boff
                        plain[sl] = rr
                        rel_flat[sl] = dd[sel]
                        rec_flat[sl] = rc[sel]
                        # high-half padding must index within the high slice
                else:
                    n = r.shape[0]
                    sl = slice(base, base + n)
                    idx_flat[sl] = r
                    plain[sl] = r
                    rel_flat[sl] = dd
                    rec_flat[sl] = rc
            self.idx16[c] = wrap_idx16(idx_flat)
            self.plain_idx[c] = plain
            self.dstrel[c] = rel_flat.reshape(self.T3, P).T
            self.recipe[c] = rec_flat.reshape(self.T3, P).T


def _pad_table(emb, remap, s_total):
    out = np.zeros((s_total, H), dtype=BF16)
    out[remap] = np.asarray(emb, dtype=F32).astype(BF16)
    return out


# ----------------------------------------------------------------------------
# Device program
# ----------------------------------------------------------------------------

class _StageDone(Exception):
    pass


def build_program(c_dt, c_td, ch3_dt, tlo_dt, ch3_td, reps=1):
    """reps > 1 chains `reps` serialized copies of the full kernel body
    (each rep's layer-1 inputs take a zero-scaled data dependency on the
    previous rep's final AllReduce output), for slope-based HW timing."""
    s_cell, s_drug = (c_dt + 1) * P, (c_td + 1) * P
    dt_bf, dt_f32, dt_i16 = mybir.dt.bfloat16, mybir.dt.float32, mybir.dt.int16
    assert NCORES * s_cell <= SPLIT, "cell table exceeds int16 gather range"
    assert reps == 1 or STAGE == 5

    nc = bacc.Bacc("TRN2", target_bir_lowering=False, debug=False,
                   enable_asserts=False, num_devices=NCORES)

    def ein(name, shape, dtype):
        return nc.dram_tensor(name, shape, dtype, kind="ExternalInput")

    tbl_drug1 = ein("tbl_drug", [NCORES * s_drug, H], dt_bf)
    tbl_cell1 = ein("tbl_cell", [NCORES * s_cell, H], dt_bf)
    slab0_drug = ein("slab0_drug", [s_drug, H], dt_bf)
    slab0_cell = ein("slab0_cell", [s_cell, H], dt_bf)
    idx_dt_d = ein("idx_dt", [P, c_dt * ch3_dt * 8], dt_i16)
    rel_dt_d = ein("rel_dt", [P, c_dt * ch3_dt], dt_f32)
    rec_dt_d = ein("rec_dt", [P, c_dt * ch3_dt], dt_f32)
    idx_td_d = ein("idx_td", [P, c_td * ch3_td * 8], dt_i16)
    rel_td_d = ein("rel_td", [P, c_td * ch3_td], dt_f32)
    rec_td_d = ein("rec_td", [P, c_td * ch3_td], dt_f32)
    w_dt_t_d = ein("w_dt_t", [H, H], dt_bf)
    w_td_t_d = ein("w_td_t", [H, H], dt_bf)
    iota_d = ein("iota", [P, P], dt_bf)
    wd_b_d = ein("wd_b", [P, H], dt_bf)
    wc_b_d = ein("wc_b", [P, H], dt_bf)
    wfb_d = ein("wfb", [P, 1], dt_f32)
    ro_d_d = ein("ro_d_idx", [P, RO_J * 8], dt_i16)
    ro_c_d = ein("ro_c_idx", [P, RO_J * 8], dt_i16)
    out_d = nc.dram_tensor("out", [P, RO_J], dt_f32, kind="ExternalOutput")
    tbl_cell2 = nc.dram_tensor("tbl_cell2", [NCORES * s_cell, H], dt_bf,
                               addr_space="Shared")
    tbl_drug2 = nc.dram_tensor("tbl_drug2", [NCORES * s_drug, H], dt_bf,
                               addr_space="Shared")
    ar_out = nc.dram_tensor("ar_out", [P, RO_J], dt_f32, addr_space="Shared")

    with tile.TileContext(nc) as tc, ExitStack() as stk:
      def _body():
        consts = stk.enter_context(tc.tile_pool(name="consts", bufs=1))

        def load_const(dram, shape, dtype, name):
            t = consts.tile(shape, dtype, tag=name)
            nc.sync.dma_start(out=t[:], in_=dram[:, :])
            return t

        idx_dt = load_const(idx_dt_d, [P, c_dt * ch3_dt * 8], dt_i16, "idx_dt")
        rel_dt = load_const(rel_dt_d, [P, c_dt * ch3_dt], dt_f32, "rel_dt")
        rec_dt = load_const(rec_dt_d, [P, c_dt * ch3_dt], dt_f32, "rec_dt")
        idx_td = load_const(idx_td_d, [P, c_td * ch3_td * 8], dt_i16, "idx_td")
        rel_td = load_const(rel_td_d, [P, c_td * ch3_td], dt_f32, "rel_td")
        rec_td = load_const(rec_td_d, [P, c_td * ch3_td], dt_f32, "rec_td")
        w_dt_t = load_const(w_dt_t_d, [H, H], dt_bf, "w_dt_t")
        w_td_t = load_const(w_td_t_d, [H, H], dt_bf, "w_td_t")
        iota = load_const(iota_d, [P, P], dt_bf, "iota")
        wd_b = load_const(wd_b_d, [P, H], dt_bf, "wd_b")
        wc_b = load_const(wc_b_d, [P, H], dt_bf, "wc_b")
        wfb = load_const(wfb_d, [P, 1], dt_f32, "wfb")
        eps_t = consts.tile([P, 1], dt_f32, tag="eps_t")
        nc.vector.memset(eps_t[:], EPS)

        def load_slab(dram, C, name):
            t = consts.tile([P, (C + 1) * P], dt_bf, tag=name)
            nc.sync.dma_start(
                out=t[:].rearrange("p (k f) -> p k f", f=H),
                in_=dram[:, :].rearrange("(k p) f -> k p f", p=P)
                .rearrange("k p f -> p k f"),
            )
            return t

        slab_cell_old = load_slab(slab0_cell, c_dt, "slab_cell_old")
        slab_drug_old = load_slab(slab0_drug, c_td, "slab_drug_old")

        work = stk.enter_context(tc.tile_pool(name="work", bufs=4))
        spool = stk.enter_context(tc.tile_pool(name="spool", bufs=6))
        psum = stk.enter_context(tc.tile_pool(name="psum", bufs=3, space="PSUM"))
        epil = stk.enter_context(tc.tile_pool(name="epil", bufs=2))

        def relation_pass(tbl_ap, idx_sb, rel_sb, rec_sb, w_t_sb, slab_old,
                          C, ch3, t_lo, split_rows, tag):
            """One message-passing direction; returns SBUF slab_new (bf16)."""
            sx_tag = "sx_dt" if tag[0] == "c" else "sx_td"
            slab_x = consts.tile([P, C * P], dt_f32, tag=sx_tag)
            xsum = consts.tile([P, C], dt_f32, tag=f"xsum_{tag}")
            ssq = consts.tile([P, C], dt_f32, tag=f"ssq_{tag}")
            slab_new = consts.tile([P, (C + 1) * P], dt_bf, tag=f"snew_{tag}")
            nc.vector.memset(slab_new[:, C * P:(C + 1) * P], 0)

            t_hi = ch3 - t_lo
            for k in range(C):
                xg = work.tile([P, ch3 * P], dt_bf, tag="xg")
                xg3 = xg[:].rearrange("p (t f) -> p t f", f=H)
                ib = k * ch3 * 8
                if split_rows is None:
                    nc.gpsimd.dma_gather(
                        xg3, tbl_ap, idx_sb[:, ib:ib + ch3 * 8],
                        ch3 * P, ch3 * P, H, single_packet=False)
                else:
                    nc.gpsimd.dma_gather(
                        xg3[:, :t_lo, :], tbl_ap[:split_rows, :],
                        idx_sb[:, ib:ib + t_lo * 8],
                        t_lo * P, t_lo * P, H, single_packet=False)
                    nc.gpsimd.dma_gather(
                        xg3[:, t_lo:, :], tbl_ap[split_rows:, :],
                        idx_sb[:, ib + t_lo * 8:ib + ch3 * 8],
                        t_hi * P, t_hi * P, H, single_packet=False)
                aggt = psum.tile([P, P], dt_f32, tag="aggt")
                for j in range(ch3):
                    t = k * ch3 + j
                    s_t = spool.tile([P, P], dt_bf, tag="s_t")
                    nc.vector.tensor_scalar(
                        out=s_t[:], in0=iota[:],
                        scalar1=rel_sb[:, t:t + 1],
                        scalar2=rec_sb[:, t:t + 1],
                        op0=mybir.AluOpType.is_equal,
                        op1=mybir.AluOpType.mult)
                    nc.tensor.matmul(
                        out=aggt[:], lhsT=xg[:, j * P:(j + 1) * P],
                        rhs=s_t[:], start=(j == 0), stop=(j == ch3 - 1))
                aggt_sb = work.tile([P, P], dt_bf, tag="aggt_sb")
                nc.vector.tensor_copy(out=aggt_sb[:], in_=aggt[:])
                y_ps = psum.tile([P, P], dt_f32, tag="y_ps")
                nc.tensor.matmul(out=y_ps[:], lhsT=aggt_sb[:], rhs=w_t_sb[:],
                                 start=True, stop=True)
                nc.vector.scalar_tensor_tensor(
                    out=slab_x[:, k * P:(k + 1) * P], in0=y_ps[:], scalar=1.0,
                    in1=slab_old[:, k * P:(k + 1) * P],
                    op0=mybir.AluOpType.mult, op1=mybir.AluOpType.add,
                    accum_out=xsum[:, k:k + 1])
                sq_tr = epil.tile([P, P], dt_f32, tag="sq_tr")
                nc.scalar.activation(
                    out=sq_tr[:], in_=slab_x[:, k * P:(k + 1) * P],
                    func=mybir.ActivationFunctionType.Square,
                    accum_out=ssq[:, k:k + 1])

            mu_neg = consts.tile([P, C], dt_f32, tag=f"mneg_{tag}")
            nc.vector.tensor_scalar(
                out=mu_neg[:], in0=xsum[:], scalar1=-1.0 / H, scalar2=None,
                op0=mybir.AluOpType.mult)
            mu2 = consts.tile([P, C], dt_f32, tag=f"mu2_{tag}")
            nc.vector.tensor_tensor(out=mu2[:], in0=mu_neg[:], in1=mu_neg[:],
                                    op=mybir.AluOpType.mult)
            var = consts.tile([P, C], dt_f32, tag=f"var_{tag}")
            nc.vector.scalar_tensor_tensor(
                out=var[:], in0=ssq[:], scalar=1.0 / H, in1=mu2[:],
                op0=mybir.AluOpType.mult, op1=mybir.AluOpType.subtract)
            std = consts.tile([P, C], dt_f32, tag=f"std_{tag}")
            nc.scalar.activation(out=std[:], in_=var[:],
                                 func=mybir.ActivationFunctionType.Sqrt,
                                 bias=eps_t[:, :1])
            rstd = consts.tile([P, C], dt_f32, tag=f"rstd_{tag}")
            nc.vector.reciprocal(out=rstd[:], in_=std[:])
            bvec = consts.tile([P, C], dt_f32, tag=f"bvec_{tag}")
            nc.vector.tensor_tensor(out=bvec[:], in0=mu_neg[:], in1=rstd[:],
                                    op=mybir.AluOpType.mult)
            for k in range(C):
                nc.scalar.activation(
                    out=slab_new[:, k * P:(k + 1) * P],
                    in_=slab_x[:, k * P:(k + 1) * P],
                    func=mybir.ActivationFunctionType.Relu,
                    bias=bvec[:, k:k + 1], scale=rstd[:, k:k + 1])
            return slab_new

        def slab_to_dram(slab_sb, C, name):
            d, _f = tc.tile([(C + 1) * P, H], dt_bf, space="DRAM", name=name)
            stk.callback(_f)
            nc.sync.dma_start(
                out=d[:].rearrange("(k p) f -> k p f", p=P)
                .rearrange("k p f -> p k f"),
                in_=slab_sb[:].rearrange("p (k f) -> p k f", f=H))
            return d

        rg = [list(range(NCORES))]
        dt_split = SPLIT if tlo_dt < ch3_dt else None

        def dbg_out(slab_sb):
            probe = consts.tile([P, RO_J], dt_f32, tag="probe")
            nc.vector.tensor_copy(out=probe[:], in_=slab_sb[:, :RO_J])
            nc.sync.dma_start(out=out_d[:, :], in_=probe[:])

        ro_d = load_const(ro_d_d, [P, RO_J * 8], dt_i16, "ro_d")
        ro_c = load_const(ro_c_d, [P, RO_J * 8], dt_i16, "ro_c")
        prob = None  # previous rep's output, serialization token

        for rep in range(reps):
          if rep == 0:
            cell_in, drug_in = slab_cell_old, slab_drug_old
          else:
            # zero-scaled dependency on the previous rep's final output:
            # rep r+1's layer-1 inputs = slab_old + 0 * prob[r]
            zt = epil.tile([P, 1], dt_f32, tag="zt")
            nc.vector.tensor_scalar(
                out=zt[:], in0=prob[:, 0:1], scalar1=0.0, scalar2=None,
                op0=mybir.AluOpType.mult)
            cell_in = consts.tile([P, s_cell], dt_bf, tag="cell_dep")
            drug_in = consts.tile([P, s_drug], dt_bf, tag="drug_dep")
            for dst, src in ((cell_in, slab_cell_old),
                             (drug_in, slab_drug_old)):
                nc.vector.tensor_scalar(
                    out=dst[:], in0=src[:], scalar1=zt[:, 0:1], scalar2=None,
                    op0=mybir.AluOpType.add)

          # ---- layer 1 ----
          slab_cell_1 = relation_pass(tbl_drug1[:, :], idx_dt, rel_dt, rec_dt,
                                      w_dt_t, cell_in, c_dt, ch3_dt,
                                      tlo_dt, dt_split, "c1")
          if STAGE == 1:
            dbg_out(slab_cell_1)
          if STAGE < 2:
            return
          slab_cell_1d = slab_to_dram(slab_cell_1, c_dt, "slab_cell_1d")
          nc.gpsimd.collective_compute(
              "AllGather", mybir.AluOpType.bypass, replica_groups=rg,
              ins=[slab_cell_1d[:]], outs=[tbl_cell2[:, :]])

          slab_drug_1 = relation_pass(tbl_cell1[:, :], idx_td, rel_td, rec_td,
                                      w_td_t, drug_in, c_td, ch3_td,
                                      ch3_td, None, "d1")
          if STAGE == 2:
            dbg_out(slab_drug_1)
          if STAGE < 3:
            return

          slab_drug_1d = slab_to_dram(slab_drug_1, c_td, "slab_drug_1d")
          nc.gpsimd.collective_compute(
              "AllGather", mybir.AluOpType.bypass, replica_groups=rg,
              ins=[slab_drug_1d[:]], outs=[tbl_drug2[:, :]])

          if STAGE == 3:
            probe_b = consts.tile([P, RO_J], dt_bf, tag="probe_b")
            nc.sync.dma_start(out=probe_b[:], in_=tbl_cell2[0:P, 0:RO_J])
            dbg_out(probe_b)
          if STAGE < 4:
            return

          # ---- layer 2 (td first: needs the cell table, gathered first) ----
          slab_drug_2 = relation_pass(tbl_cell2[:, :], idx_td, rel_td, rec_td,
                                      w_td_t, slab_drug_1, c_td, ch3_td,
                                      ch3_td, None, "d2")
          slab_cell_2 = relation_pass(tbl_drug2[:, :], idx_dt, rel_dt, rec_dt,
                                      w_dt_t, slab_cell_1, c_dt, ch3_dt,
                                      tlo_dt, dt_split, "c2")

          if STAGE == 4:
            dbg_out(slab_drug_2)
          if STAGE < 5:
            return

          # ---- readout ----
          slab_drug_2d = slab_to_dram(slab_drug_2, c_td, "slab_drug_2d")
          slab_cell_2d = slab_to_dram(slab_cell_2, c_dt, "slab_cell_2d")

          xd = consts.tile([P, RO_J * H], dt_bf, tag="xd")
          nc.gpsimd.dma_gather(
              xd[:].rearrange("p (j f) -> p j f", f=H), slab_drug_2d[:],
              ro_d[:], NB, NB, H, single_packet=False)
          xc = consts.tile([P, RO_J * H], dt_bf, tag="xc")
          nc.gpsimd.dma_gather(
              xc[:].rearrange("p (j f) -> p j f", f=H), slab_cell_2d[:],
              ro_c[:], NB, NB, H, single_packet=False)

          sd = consts.tile([P, RO_J], dt_f32, tag="sd")
          sc = consts.tile([P, RO_J], dt_f32, tag="sc")
          for j in range(RO_J):
            for x_t, w_t, s_t in ((xd, wd_b, sd), (xc, wc_b, sc)):
                mres = epil.tile([P, H], dt_f32, tag="ro_mres")
                nc.vector.tensor_tensor(out=mres[:],
                                        in0=x_t[:, j * H:(j + 1) * H],
                                        in1=w_t[:], op=mybir.AluOpType.mult)
                nc.vector.tensor_reduce(out=s_t[:, j:j + 1], in_=mres[:],
                                        axis=mybir.AxisListType.X,
                                        op=mybir.AluOpType.add)
          part = consts.tile([P, RO_J], dt_f32, tag="part")
          nc.vector.tensor_tensor(out=part[:], in0=sd[:], in1=sc[:],
                                  op=mybir.AluOpType.add)
          ar_in, _f3 = tc.tile([P, RO_J], dt_f32, space="DRAM", name="ar_in")
          stk.callback(_f3)
          nc.sync.dma_start(out=ar_in[:], in_=part[:])
          nc.gpsimd.collective_compute(
              "AllReduce", mybir.AluOpType.add, replica_groups=rg,
              ins=[ar_in[:]], outs=[ar_out[:, :]])
          logit = consts.tile([P, RO_J], dt_f32, tag="logit")
          nc.sync.dma_start(out=logit[:], in_=ar_out[:, :])
          prob = consts.tile([P, RO_J], dt_f32, tag="prob")
          nc.scalar.activation(out=prob[:], in_=logit[:],
                               func=mybir.ActivationFunctionType.Sigmoid,
                               bias=wfb[:, :1])
          nc.sync.dma_start(out=out_d[:, :], in_=prob[:])
      _body()

    nc.compile()
    return nc


# ----------------------------------------------------------------------------
# Entry point
# ----------------------------------------------------------------------------

_CACHE = {}


def _prepare(inputs):
    dt = RelPrep(inputs["edge_dt_src"], inputs["edge_dt_dst"], NUM_CELL, CH_DT)
    td = RelPrep(inputs["edge_td_src"], inputs["edge_td_dst"], NUM_DRUG, CH_TD)
    # dt gathers drug rows (laid out by td's chunks); td gathers cell rows
    dt.finalize(td.remap, SPLIT if NCORES * td.S > SPLIT else None)
    td.finalize(dt.remap, SPLIT if NCORES * dt.S > SPLIT else None)

    tbl_drug = _pad_table(inputs["emb_drug"], td.remap, NCORES * td.S)
    tbl_cell = _pad_table(inputs["emb_cell"], dt.remap, NCORES * dt.S)

    w_dt_t = np.ascontiguousarray(np.asarray(inputs["W_dt"], dtype=F32).T).astype(BF16)
    w_td_t = np.ascontiguousarray(np.asarray(inputs["W_td"], dtype=F32).T).astype(BF16)
    iota = np.tile(np.arange(P, dtype=F32), (P, 1)).astype(BF16)
    wf = np.asarray(inputs["W_final_w"], dtype=F32)[0]
    wd_b = np.tile(wf[:H], (P, 1)).astype(BF16)
    wc_b = np.tile(wf[H:], (P, 1)).astype(BF16)
    wfb = np.full((P, 1), np.asarray(inputs["W_final_b"], dtype=F32)[0], dtype=F32)

    drug_ids = np.asarray(inputs["drug_ids"], dtype=np.int64)
    cell_ids = np.asarray(inputs["cell_ids"], dtype=np.int64)
    gd = td.remap[drug_ids].reshape(P, RO_J)
    gc = dt.remap[cell_ids].reshape(P, RO_J)

    in_maps = []
    for c in range(NCORES):
        own_d = (gd // td.S) == c
        own_c = (gc // dt.S) == c
        ro_d = np.where(own_d, gd - c * td.S, td.C * P)
        ro_c = np.where(own_c, gc - c * dt.S, dt.C * P)
        # dma_gather puts flat position i at dst[i%128, i//128]; batch element
        # b = p*RO_J + j must land at [p, j] => use position i = j*128 + p.
        ro_d_flat = ro_d.T.reshape(-1)
        ro_c_flat = ro_c.T.reshape(-1)
        in_maps.append({
            "tbl_drug": tbl_drug,
            "tbl_cell": tbl_cell,
            "slab0_drug": tbl_drug[c * td.S:(c + 1) * td.S],
            "slab0_cell": tbl_cell[c * dt.S:(c + 1) * dt.S],
            "idx_dt": dt.idx16[c],
            "rel_dt": dt.dstrel[c].astype(F32),
            "rec_dt": dt.recipe[c].astype(F32),
            "idx_td": td.idx16[c],
            "rel_td": td.dstrel[c].astype(F32),
            "rec_td": td.recipe[c].astype(F32),
            "w_dt_t": w_dt_t,
            "w_td_t": w_td_t,
            "iota": iota,
            "wd_b": wd_b,
            "wc_b": wc_b,
            "wfb": wfb,
            "ro_d_idx": wrap_idx16(ro_d_flat),
            "ro_c_idx": wrap_idx16(ro_c_flat),
        })
    return dt, td, in_maps


def kernel(**inputs) -> np.ndarray:
    dt, td, in_maps = _prepare(inputs)
    key = (dt.C, td.C, dt.ch3, dt.t_lo, td.ch3)
    if key not in _CACHE:
        _CACHE[key] = build_program(dt.C, td.C, dt.ch3, dt.t_lo, td.ch3)
    nc = _CACHE[key]
    res = run_bass_kernel_spmd(nc, in_maps, core_ids=list(range(NCORES)))
    out = res.results[0]["out"]
    return np.asarray(out, dtype=np.float32).reshape(NB)

